# revision 1
# baseline (speedup 1.0000x reference)
"""Pairwise squared L2 distance (retrieval KNN) on 8 TRN2 NeuronCores.

dist[i, j] = ||x_i||^2 + ||y_j||^2 - 2 * <x_i, y_j>

Sharding: rows of x are split across the 8 cores (data-parallel over n);
y is replicated. Each core computes a [1024, 8192] slab of the distance
matrix.

The cross term x @ y^T runs as an fp16 hi/lo split GEMM (x ~ xh + xl,
y ~ yh + yl; cross = xh@yh + xh@yl + xl@yh, accumulated in fp32 PSUM),
giving ~5e-7 relative error at full PE rate (fp32/fp32r matmuls are
2-4x slower on TRN2). The norm terms ride the epilogue: ScalarE
computes -2*psum + x_sq (per-partition bias), VectorE adds a broadcast
y_sq tile (built once on-chip by gpsimd partition_broadcast, exact
fp32), and 1 MiB stores stream the result out. Column groups are the
outer loop so compute starts after the first 1 MiB of y has landed.
Inputs are laid out host-side (transposes, fp16 split, norm vectors) so
the device does no transposes.
"""

import numpy as np

import concourse.bass as bass
import concourse.mybir as mybir
import concourse.tile as tile
from concourse import bacc
from concourse.bass import ts
from concourse.bass_utils import run_bass_kernel_spmd

N, M, D = 8192, 8192, 128
NCORES = 8
SLAB = N // NCORES  # 1024 rows of x per core
P = 128  # partitions / m-chunk height
MCH = SLAB // P  # 8 m-chunks per core
NT = 512  # matmul free-dim tile (one fp32 PSUM bank)
GW = 4  # n-chunks per PSUM group (4 banks = 8 KiB/partition)
GCOLS = GW * NT  # 2048
NG = M // GCOLS  # 4 column groups
LW = 2048  # y load-chunk width
YC = M // LW  # 4 load chunks
NCH = M // NT  # 16 n-chunks

_f32 = mybir.dt.float32
_f16 = mybir.dt.float16
_IDENT = mybir.ActivationFunctionType.Identity

_compiled_nc = None


def _build():
    """Build + compile the single-core Bass program (SPMD across 8 cores)."""
    nc = bacc.Bacc(
        "TRN2",
        target_bir_lowering=False,
        debug=False,
        enable_asserts=False,
        num_devices=NCORES,
    )
    xh = nc.dram_tensor("xh", [D, SLAB], _f16, kind="ExternalInput").ap()
    xl = nc.dram_tensor("xl", [D, SLAB], _f16, kind="ExternalInput").ap()
    yh = nc.dram_tensor("yh", [D, M], _f16, kind="ExternalInput").ap()
    yl = nc.dram_tensor("yl", [D, M], _f16, kind="ExternalInput").ap()
    xsq = nc.dram_tensor("xsq", [P, MCH], _f32, kind="ExternalInput").ap()
    ysq = nc.dram_tensor("ysq", [1, M], _f32, kind="ExternalInput").ap()
    dist = nc.dram_tensor("dist", [SLAB, M], _f32, kind="ExternalOutput").ap()

    with tile.TileContext(nc) as tc:
        with (
            tc.tile_pool(name="consts", bufs=1) as cpool,
            tc.tile_pool(name="psum", bufs=2, space="PSUM") as pspool,
            tc.tile_pool(name="abuf", bufs=4) as apool,
            tc.tile_pool(name="obuf", bufs=4) as opool,
        ):
            # First-group inputs lead so the PE can start ASAP: y chunk 0 on
            # the SP ring, then x + the epilogue vectors, then the rest of y.
            yh_sb = cpool.tile([D, M], _f16)
            yl_sb = cpool.tile([D, M], _f16)
            nc.sync.dma_start(yh_sb[:, ts(0, LW)], yh[:, ts(0, LW)])
            nc.sync.dma_start(yl_sb[:, ts(0, LW)], yl[:, ts(0, LW)])
            xh_sb = cpool.tile([D, SLAB], _f16)
            nc.sync.dma_start(xh_sb[:], xh[:])
            xl_sb = cpool.tile([D, SLAB], _f16)
            nc.sync.dma_start(xl_sb[:], xl[:])
            ysq_row = cpool.tile([1, M], _f32)
            nc.sync.dma_start(ysq_row[:], ysq[:])
            xsq_sb = cpool.tile([P, MCH], _f32)
            nc.sync.dma_start(xsq_sb[:], xsq[:])
            for c in range(1, YC):
                nc.sync.dma_start(yh_sb[:, ts(c, LW)], yh[:, ts(c, LW)])
                nc.sync.dma_start(yl_sb[:, ts(c, LW)], yl[:, ts(c, LW)])

            # ysq_b[p, j] = y_sq[j], exact fp32, built on the otherwise-idle
            # GpSimd engine in group-sized chunks.
            ysq_b = cpool.tile([P, M], _f32)
            for c in range(YC):
                nc.gpsimd.partition_broadcast(
                    ysq_b[:, ts(c, LW)], ysq_row[0:1, ts(c, LW)]
                )

            def emit_block(mc, j0, w):
                """One [128, w*NT] output block: 3*w matmuls + epilogue + store."""
                xh_w = xh_sb[:, ts(mc, P)]
                xl_w = xl_sb[:, ts(mc, P)]
                cols = w * NT
                ps = pspool.tile([P, cols], _f32, tag="ps")
                # Weight-reuse order: xh held for the first 2*w matmuls,
                # then xl for w.
                for jj in range(w):
                    nc.tensor.matmul(
                        ps[:, ts(jj, NT)],
                        xh_w,
                        yh_sb[:, ts(j0 + jj, NT)],
                        start=True,
                        stop=False,
                    )
                for jj in range(w):
                    nc.tensor.matmul(
                        ps[:, ts(jj, NT)],
                        xh_w,
                        yl_sb[:, ts(j0 + jj, NT)],
                        start=False,
                        stop=False,
                    )
                for jj in range(w):
                    nc.tensor.matmul(
                        ps[:, ts(jj, NT)],
                        xl_w,
                        yh_sb[:, ts(j0 + jj, NT)],
                        start=False,
                        stop=True,
                    )
                # Epilogue: a = -2*psum + x_sq (ACT), out = a + y_sq (DVE)
                a = apool.tile([P, cols], _f32, tag="a")
                nc.scalar.activation(
                    a[:],
                    ps[:],
                    _IDENT,
                    bias=xsq_sb[:, mc : mc + 1],
                    scale=-2.0,
                )
                ot = opool.tile([P, cols], _f32, tag="ot")
                nc.vector.tensor_add(
                    ot[:], a[:], ysq_b[:, j0 * NT : j0 * NT + cols]
                )
                nc.sync.dma_start(
                    dist[ts(mc, P), j0 * NT : j0 * NT + cols], ot[:]
                )

            for g in range(NG):
                for mc in range(MCH):
                    emit_block(mc, g * GW, GW)

    nc.compile()
    return nc


def _get_nc():
    global _compiled_nc
    if _compiled_nc is None:
        _compiled_nc = _build()
    return _compiled_nc


def make_in_maps(x: np.ndarray, y: np.ndarray) -> list[dict[str, np.ndarray]]:
    x = np.asarray(x, dtype=np.float32)
    y = np.asarray(y, dtype=np.float32)
    x_sq = np.sum(x * x, axis=1, dtype=np.float32)
    y_sq = np.sum(y * y, axis=1, dtype=np.float32)

    xt = x.T  # [D, N]
    yt = y.T  # [D, M]
    xt_hi = xt.astype(np.float16)
    xt_lo = (xt - xt_hi.astype(np.float32)).astype(np.float16)
    yt_hi = np.ascontiguousarray(yt.astype(np.float16))
    yt_lo = np.ascontiguousarray((yt - yt_hi.astype(np.float32)).astype(np.float16))

    ysq_in = np.ascontiguousarray(y_sq.reshape(1, M))

    in_maps = []
    for c in range(NCORES):
        sl = slice(c * SLAB, (c + 1) * SLAB)
        # [P, MCH]: column mc holds x_sq for rows mc*128..mc*128+127
        xsq_in = np.ascontiguousarray(x_sq[sl].reshape(MCH, P).T)
        in_maps.append(
            {
                "xh": np.ascontiguousarray(xt_hi[:, sl]),
                "xl": np.ascontiguousarray(xt_lo[:, sl]),
                "yh": yt_hi,
                "yl": yt_lo,
                "xsq": xsq_in,
                "ysq": ysq_in,
            }
        )
    return in_maps


def kernel(x: np.ndarray, y: np.ndarray, **run_kwargs) -> np.ndarray:
    nc = _get_nc()
    in_maps = make_in_maps(x, y)
    res = run_bass_kernel_spmd(nc, in_maps, core_ids=list(range(NCORES)), **run_kwargs)
    out = np.concatenate([res.results[c]["dist"] for c in range(NCORES)], axis=0)
    if run_kwargs:
        kernel.last_results = res
    return out



# revision 2
# speedup vs baseline: 1.1564x; 1.1564x over previous
"""Pairwise squared L2 distance (retrieval KNN) on 8 TRN2 NeuronCores.

dist[i, j] = ||x_i||^2 + ||y_j||^2 - 2 * <x_i, y_j>

Sharding: rows of x are split across the 8 cores (data-parallel over n);
y is replicated. Each core computes a [1024, 8192] slab of the distance
matrix.

The kernel is memory-bound on the output stores, so the slab is computed
and stored in fp16 (dist magnitudes are 100..500 with min |dist| ~ 113,
so fp16 keeps relative error ~1e-3) and upcast to fp32 on the host.
That halves HBM store traffic vs fp32 (16 MiB/core) and sets the DMA
roofline at ~53 us/core.

The cross term runs as a single fp16 matmul (x pre-scaled by -2 on the
host, so PSUM accumulates -2<x,y> directly). The norm terms are split
across engines to keep everything under the DMA roofline:
  - even blocks: ysq rides the PE as a 1-row accumulate matmul
    (ones[1,128] x ysq[1,512]), then ScalarE writes fp16 out with the
    per-partition xsq bias.
  - odd blocks: one fused VectorE scalar_tensor_tensor:
    out = (psum + xsq) + ysq_bcast, reading the gpsimd-built [128, M]
    fp16 broadcast of ysq.
Column groups are the outer loop so compute starts after the first
0.5 MiB of y has landed. All transposes / fp16 casts / norm vectors are
prepared host-side.
"""

import numpy as np

import concourse.bass as bass
import concourse.mybir as mybir
import concourse.tile as tile
from concourse import bacc
from concourse.bass import ts
from concourse.bass_utils import run_bass_kernel_spmd

N, M, D = 8192, 8192, 128
NCORES = 8
SLAB = N // NCORES  # 1024 rows of x per core
P = 128  # partitions / m-chunk height
MCH = SLAB // P  # 8 m-chunks per core
NT = 512  # matmul free-dim tile (one fp32 PSUM bank)
GW = 4  # n-chunks per PSUM group (4 banks = 8 KiB/partition)
GCOLS = GW * NT  # 2048
NG = M // GCOLS  # 4 column groups
LW = 2048  # y load-chunk width
YC = M // LW  # 4 load chunks

_f32 = mybir.dt.float32
_f16 = mybir.dt.float16
_IDENT = mybir.ActivationFunctionType.Identity
_ADD = mybir.AluOpType.add

_compiled_nc = None


def _build():
    """Build + compile the single-core Bass program (SPMD across 8 cores)."""
    nc = bacc.Bacc(
        "TRN2",
        target_bir_lowering=False,
        debug=False,
        enable_asserts=False,
        num_devices=NCORES,
    )
    xm2 = nc.dram_tensor("xm2", [D, SLAB], _f16, kind="ExternalInput").ap()
    yh = nc.dram_tensor("yh", [D, M], _f16, kind="ExternalInput").ap()
    ysq16 = nc.dram_tensor("ysq16", [1, M], _f16, kind="ExternalInput").ap()
    xsq = nc.dram_tensor("xsq", [P, MCH], _f32, kind="ExternalInput").ap()
    ones = nc.dram_tensor("ones", [1, P], _f16, kind="ExternalInput").ap()
    dist16 = nc.dram_tensor("dist16", [SLAB, M], _f16, kind="ExternalOutput").ap()

    with tile.TileContext(nc) as tc:
        with (
            tc.tile_pool(name="consts", bufs=1) as cpool,
            tc.tile_pool(name="psum", bufs=2, space="PSUM") as pspool,
            tc.tile_pool(name="obuf", bufs=4) as opool,
        ):
            # First-group inputs lead so the PE can start ASAP: y chunk 0,
            # then x + the epilogue vectors, then the rest of y.
            yh_sb = cpool.tile([D, M], _f16)
            nc.sync.dma_start(yh_sb[:, ts(0, LW)], yh[:, ts(0, LW)])
            xm2_sb = cpool.tile([D, SLAB], _f16)
            nc.sync.dma_start(xm2_sb[:], xm2[:])
            ysq_row = cpool.tile([1, M], _f16)
            nc.sync.dma_start(ysq_row[:], ysq16[:])
            xsq_sb = cpool.tile([P, MCH], _f32)
            nc.sync.dma_start(xsq_sb[:], xsq[:])
            ones_sb = cpool.tile([1, P], _f16)
            nc.sync.dma_start(ones_sb[:], ones[:])
            for c in range(1, YC):
                nc.sync.dma_start(yh_sb[:, ts(c, LW)], yh[:, ts(c, LW)])

            # ysq_b[p, j] = ysq16[j], built on the otherwise-idle GpSimd
            # engine in group-sized chunks (used by the odd-block epilogue).
            ysq_b = cpool.tile([P, M], _f16)
            for c in range(YC):
                nc.gpsimd.partition_broadcast(
                    ysq_b[:, ts(c, LW)], ysq_row[0:1, ts(c, LW)]
                )

            def emit_block(mc, g, pe_ysq):
                """One [128, GCOLS] output block."""
                x_w = xm2_sb[:, ts(mc, P)]
                j0 = g * GW
                cols = GCOLS
                c0 = g * GCOLS
                ps = pspool.tile([P, cols], _f32, tag="ps")
                for jj in range(GW):
                    nc.tensor.matmul(
                        ps[:, ts(jj, NT)],
                        x_w,
                        yh_sb[:, ts(j0 + jj, NT)],
                        start=True,
                        stop=not pe_ysq,
                    )
                if pe_ysq:
                    # ysq as a 1-row accumulate matmul, then ScalarE
                    # epilogue: out = psum + xsq (per-partition bias).
                    for jj in range(GW):
                        nc.tensor.matmul(
                            ps[:, ts(jj, NT)],
                            ones_sb[:],
                            ysq_row[0:1, ts(j0 + jj, NT)],
                            start=False,
                            stop=True,
                        )
                    ot = opool.tile([P, cols], _f16, tag="ot")
                    nc.scalar.activation(
                        ot[:],
                        ps[:],
                        _IDENT,
                        bias=xsq_sb[:, mc : mc + 1],
                        scale=1.0,
                    )
                else:
                    # Fused VectorE epilogue: out = (psum + xsq) + ysq_b.
                    ot = opool.tile([P, cols], _f16, tag="ot")
                    nc.vector.scalar_tensor_tensor(
                        ot[:],
                        ps[:],
                        xsq_sb[:, mc : mc + 1],
                        ysq_b[:, c0 : c0 + cols],
                        op0=_ADD,
                        op1=_ADD,
                    )
                nc.sync.dma_start(dist16[ts(mc, P), c0 : c0 + cols], ot[:])

            for g in range(NG):
                for mc in range(MCH):
                    emit_block(mc, g, pe_ysq=(mc % 2 == 0))

    nc.compile()
    return nc


def _get_nc():
    global _compiled_nc
    if _compiled_nc is None:
        _compiled_nc = _build()
    return _compiled_nc


def make_in_maps(x: np.ndarray, y: np.ndarray) -> list[dict[str, np.ndarray]]:
    x = np.asarray(x, dtype=np.float32)
    y = np.asarray(y, dtype=np.float32)
    x_sq = np.sum(x * x, axis=1, dtype=np.float32)
    y_sq = np.sum(y * y, axis=1, dtype=np.float32)

    xm2t = (-2.0 * x).T.astype(np.float16)  # [D, N]
    yt16 = np.ascontiguousarray(y.T.astype(np.float16))  # [D, M]
    ysq_in = np.ascontiguousarray(y_sq.astype(np.float16).reshape(1, M))
    ones_in = np.ones((1, P), dtype=np.float16)

    in_maps = []
    for c in range(NCORES):
        sl = slice(c * SLAB, (c + 1) * SLAB)
        # [P, MCH]: column mc holds x_sq for rows mc*128..mc*128+127
        xsq_in = np.ascontiguousarray(x_sq[sl].reshape(MCH, P).T)
        in_maps.append(
            {
                "xm2": np.ascontiguousarray(xm2t[:, sl]),
                "yh": yt16,
                "ysq16": ysq_in,
                "xsq": xsq_in,
                "ones": ones_in,
            }
        )
    return in_maps


def kernel(x: np.ndarray, y: np.ndarray, **run_kwargs) -> np.ndarray:
    nc = _get_nc()
    in_maps = make_in_maps(x, y)
    res = run_bass_kernel_spmd(nc, in_maps, core_ids=list(range(NCORES)), **run_kwargs)
    out = np.concatenate(
        [res.results[c]["dist16"] for c in range(NCORES)], axis=0
    ).astype(np.float32)
    if run_kwargs:
        kernel.last_results = res
    return out


# revision 6
# speedup vs baseline: 1.2037x; 1.0409x over previous
"""Pairwise squared L2 distance (retrieval KNN) on 8 TRN2 NeuronCores.

dist[i, j] = ||x_i||^2 + ||y_j||^2 - 2 * <x_i, y_j>

Sharding: rows of x are split across the 8 cores (data-parallel over n);
y is replicated. Each core computes a [1024, 8192] slab of the distance
matrix.

The kernel is memory-bound on the output stores, so the slab is computed
and stored in fp16 (dist magnitudes are 100..500 with min |dist| ~ 113,
so fp16 keeps relative error ~1e-3) and upcast to fp32 on the host.
That halves HBM store traffic vs fp32 (16 MiB/core) and sets the DMA
roofline at ~53 us/core.

The cross term runs as a single fp16 matmul (x pre-scaled by -2 on the
host, so PSUM accumulates -2<x,y> directly). On this part the PE
streams ~1 row/ns under the 8-core power throttle (measured 491 ns per
[128,512] fp16 matmul), so the 128 cross matmuls are the ~63 us
critical path and nothing else may ride the PE. The norm terms go on
the vector engines, balanced to stay under that:
  - 1 in 4 blocks: one fused VectorE scalar_tensor_tensor:
    out = (psum + xsq) + ysq_bcast  (fp32 PSUM read, ~2.35 us)
  - 3 in 4 blocks: ScalarE activation a = psum + xsq (per-partition
    bias, fp16 out, ~1.97 us) then a cheap all-fp16 2x-mode VectorE
    add of the ysq broadcast (~1.2 us).
ysq_bcast is the gpsimd-built [128, M] fp16 broadcast of ysq.
Column groups are the outer loop so compute starts after the first
0.5 MiB of y has landed. All transposes / fp16 casts / norm vectors are
prepared host-side.
"""

import numpy as np

import concourse.bass as bass
import concourse.mybir as mybir
import concourse.tile as tile
from concourse import bacc
from concourse.bass import ts
from concourse.bass_utils import run_bass_kernel_spmd

N, M, D = 8192, 8192, 128
NCORES = 8
SLAB = N // NCORES  # 1024 rows of x per core
P = 128  # partitions / m-chunk height
MCH = SLAB // P  # 8 m-chunks per core
NT = 512  # matmul free-dim tile (one fp32 PSUM bank)
GW = 4  # n-chunks per PSUM group (4 banks = 8 KiB/partition)
GCOLS = GW * NT  # 2048
NG = M // GCOLS  # 4 column groups
LW = 2048  # y load-chunk width
YC = M // LW  # 4 load chunks

_f32 = mybir.dt.float32
_f16 = mybir.dt.float16
_IDENT = mybir.ActivationFunctionType.Identity
_ADD = mybir.AluOpType.add

_compiled_nc = None


def _build():
    """Build + compile the single-core Bass program (SPMD across 8 cores)."""
    nc = bacc.Bacc(
        "TRN2",
        target_bir_lowering=False,
        debug=False,
        enable_asserts=False,
        num_devices=NCORES,
    )
    xm2 = nc.dram_tensor("xm2", [D, SLAB], _f16, kind="ExternalInput").ap()
    yh = nc.dram_tensor("yh", [D, M], _f16, kind="ExternalInput").ap()
    ysq16 = nc.dram_tensor("ysq16", [1, M], _f16, kind="ExternalInput").ap()
    xsq = nc.dram_tensor("xsq", [P, MCH], _f32, kind="ExternalInput").ap()
    dist16 = nc.dram_tensor("dist16", [SLAB, M], _f16, kind="ExternalOutput").ap()

    with tile.TileContext(nc) as tc:
        with (
            tc.tile_pool(name="consts", bufs=1) as cpool,
            tc.tile_pool(name="psum", bufs=2, space="PSUM") as pspool,
            tc.tile_pool(name="abuf", bufs=4) as apool,
            tc.tile_pool(name="obuf", bufs=4) as opool,
        ):
            # First-group inputs lead so the PE can start ASAP: y chunk 0,
            # then x + the epilogue vectors, then the rest of y.
            yh_sb = cpool.tile([D, M], _f16)
            nc.sync.dma_start(yh_sb[:, ts(0, LW)], yh[:, ts(0, LW)])
            xm2_sb = cpool.tile([D, SLAB], _f16)
            nc.sync.dma_start(xm2_sb[:], xm2[:])
            ysq_row = cpool.tile([1, M], _f16)
            nc.sync.dma_start(ysq_row[:], ysq16[:])
            xsq_sb = cpool.tile([P, MCH], _f32)
            nc.sync.dma_start(xsq_sb[:], xsq[:])
            for c in range(1, YC):
                nc.sync.dma_start(yh_sb[:, ts(c, LW)], yh[:, ts(c, LW)])

            # ysq_b[p, j] = ysq16[j], built on the otherwise-idle GpSimd
            # engine in group-sized chunks (used by the odd-block epilogue).
            ysq_b = cpool.tile([P, M], _f16)
            for c in range(YC):
                nc.gpsimd.partition_broadcast(
                    ysq_b[:, ts(c, LW)], ysq_row[0:1, ts(c, LW)]
                )

            def emit_block(mc, g, fused_dve):
                """One [128, GCOLS] output block."""
                x_w = xm2_sb[:, ts(mc, P)]
                j0 = g * GW
                cols = GCOLS
                c0 = g * GCOLS
                ps = pspool.tile([P, cols], _f32, tag="ps")
                for jj in range(GW):
                    nc.tensor.matmul(
                        ps[:, ts(jj, NT)],
                        x_w,
                        yh_sb[:, ts(j0 + jj, NT)],
                        start=True,
                        stop=True,
                    )
                ot = opool.tile([P, cols], _f16, tag="ot")
                if fused_dve:
                    # Fused VectorE epilogue: out = (psum + xsq) + ysq_b.
                    nc.vector.scalar_tensor_tensor(
                        ot[:],
                        ps[:],
                        xsq_sb[:, mc : mc + 1],
                        ysq_b[:, c0 : c0 + cols],
                        op0=_ADD,
                        op1=_ADD,
                    )
                else:
                    # ScalarE: a = psum + xsq (frees PSUM), then an all-fp16
                    # 2x-mode VectorE add of the ysq broadcast.
                    a = apool.tile([P, cols], _f16, tag="a")
                    nc.scalar.activation(
                        a[:],
                        ps[:],
                        _IDENT,
                        bias=xsq_sb[:, mc : mc + 1],
                        scale=1.0,
                    )
                    nc.vector.tensor_add(ot[:], a[:], ysq_b[:, c0 : c0 + cols])
                nc.sync.dma_start(dist16[ts(mc, P), c0 : c0 + cols], ot[:])

            for g in range(NG):
                for mc in range(MCH):
                    emit_block(mc, g, fused_dve=(mc % 4 == 0))

    nc.compile()
    return nc


def _get_nc():
    global _compiled_nc
    if _compiled_nc is None:
        _compiled_nc = _build()
    return _compiled_nc


def make_in_maps(x: np.ndarray, y: np.ndarray) -> list[dict[str, np.ndarray]]:
    x = np.asarray(x, dtype=np.float32)
    y = np.asarray(y, dtype=np.float32)
    x_sq = np.sum(x * x, axis=1, dtype=np.float32)
    y_sq = np.sum(y * y, axis=1, dtype=np.float32)

    xm2t = (-2.0 * x).T.astype(np.float16)  # [D, N]
    yt16 = np.ascontiguousarray(y.T.astype(np.float16))  # [D, M]
    ysq_in = np.ascontiguousarray(y_sq.astype(np.float16).reshape(1, M))

    in_maps = []
    for c in range(NCORES):
        sl = slice(c * SLAB, (c + 1) * SLAB)
        # [P, MCH]: column mc holds x_sq for rows mc*128..mc*128+127
        xsq_in = np.ascontiguousarray(x_sq[sl].reshape(MCH, P).T)
        in_maps.append(
            {
                "xm2": np.ascontiguousarray(xm2t[:, sl]),
                "yh": yt16,
                "ysq16": ysq_in,
                "xsq": xsq_in,
            }
        )
    return in_maps


def kernel(x: np.ndarray, y: np.ndarray, **run_kwargs) -> np.ndarray:
    nc = _get_nc()
    in_maps = make_in_maps(x, y)
    res = run_bass_kernel_spmd(nc, in_maps, core_ids=list(range(NCORES)), **run_kwargs)
    out = np.concatenate(
        [res.results[c]["dist16"] for c in range(NCORES)], axis=0
    ).astype(np.float32)
    if run_kwargs:
        kernel.last_results = res
    return out


# revision 8
# speedup vs baseline: 1.3849x; 1.1506x over previous
"""Pairwise squared L2 distance (retrieval KNN) on 8 TRN2 NeuronCores.

dist[i, j] = ||x_i||^2 + ||y_j||^2 - 2 * <x_i, y_j>

Sharding: rows of x are split across the 8 cores (data-parallel over n);
y is replicated. Each core computes a [1024, 8192] slab of the distance
matrix.

The kernel is memory-bound on the output stores (measured ~305 GB/s
per-core HBM under 8-core load), so the slab is computed and stored in
fp16 (dist magnitudes are 85..500, fp16 keeps relative error ~1e-3
against the 2e-2 gate) and upcast to fp32 on the host. That halves HBM
store traffic vs fp32 (16 MiB/core) and sets the DMA roofline at
~63 us/core.

The cross term runs as a single fp16 matmul per tile (x pre-scaled by
-2 on the host, so PSUM accumulates -2<x,y> directly). The PE streams
rows at 216-427 ns per [128,512] tile (power-throttle duty cycling), so
128 matmuls ~ 41 us — under the DMA roofline as long as the PE never
stalls. To keep every engine streaming, PSUM is split into 2-bank
groups (1024 cols) with 4 buffers in flight; the norm-term epilogue is
balanced across ScalarE and VectorE per 1024-wide group:
  - 1 in 4 groups: fused VectorE scalar_tensor_tensor
    out = (psum + xsq) + ysq_bcast  (~1.2 us)
  - 3 in 4 groups: ScalarE a = psum + xsq (per-partition bias, fp16
    out, ~1.0 us) then an all-fp16 2x-mode VectorE add of the ysq
    broadcast (~0.64 us).
Two adjacent groups write halves of one [128, 2048] fp16 tile so
stores stay 1 MiB-per-2-groups. ysq_bcast is the gpsimd-built [128, M]
fp16 broadcast of ysq. Initial loads are issued from three different
engine queues in parallel to shorten the head. All transposes / fp16
casts / norm vectors are prepared host-side.
"""

import numpy as np

import concourse.bass as bass
import concourse.mybir as mybir
import concourse.tile as tile
from concourse import bacc
from concourse.bass import ts
from concourse.bass_utils import run_bass_kernel_spmd

N, M, D = 8192, 8192, 128
NCORES = 8
SLAB = N // NCORES  # 1024 rows of x per core
P = 128  # partitions / m-chunk height
MCH = SLAB // P  # 8 m-chunks per core
NT = 512  # matmul free-dim tile (one fp32 PSUM bank)
GW = 2  # n-chunks per PSUM group (2 banks = 4 KiB/partition)
GCOLS = GW * NT  # 1024
SCOLS = 2048  # store tile width (two PSUM groups)
NG = M // SCOLS  # 4 store column groups
LW = 2048  # y load-chunk width
YC = M // LW  # 4 load chunks

_f32 = mybir.dt.float32
_f16 = mybir.dt.float16
_IDENT = mybir.ActivationFunctionType.Identity
_ADD = mybir.AluOpType.add

_compiled_nc = None


def _build():
    """Build + compile the single-core Bass program (SPMD across 8 cores)."""
    nc = bacc.Bacc(
        "TRN2",
        target_bir_lowering=False,
        debug=False,
        enable_asserts=False,
        num_devices=NCORES,
    )
    xm2 = nc.dram_tensor("xm2", [D, SLAB], _f16, kind="ExternalInput").ap()
    yh = nc.dram_tensor("yh", [D, M], _f16, kind="ExternalInput").ap()
    ysq16 = nc.dram_tensor("ysq16", [1, M], _f16, kind="ExternalInput").ap()
    xsq = nc.dram_tensor("xsq", [P, MCH], _f32, kind="ExternalInput").ap()
    dist16 = nc.dram_tensor("dist16", [SLAB, M], _f16, kind="ExternalOutput").ap()

    with tile.TileContext(nc) as tc:
        with (
            tc.tile_pool(name="consts", bufs=1) as cpool,
            tc.tile_pool(name="psum", bufs=4, space="PSUM") as pspool,
            tc.tile_pool(name="abuf", bufs=4) as apool,
            tc.tile_pool(name="obuf", bufs=4) as opool,
        ):
            # First-group inputs lead so the PE can start ASAP. Issue the
            # first loads from three different engine queues in parallel
            # (HWDGE issue is ~600 ns serialized per queue).
            yh_sb = cpool.tile([D, M], _f16)
            nc.sync.dma_start(yh_sb[:, ts(0, GCOLS)], yh[:, ts(0, GCOLS)])
            xm2_sb = cpool.tile([D, SLAB], _f16)
            nc.scalar.dma_start(xm2_sb[:], xm2[:])
            ysq_row = cpool.tile([1, M], _f16)
            nc.gpsimd.dma_start(ysq_row[:], ysq16[:])
            xsq_sb = cpool.tile([P, MCH], _f32)
            nc.scalar.dma_start(xsq_sb[:], xsq[:])
            nc.sync.dma_start(
                yh_sb[:, GCOLS : 2 * GCOLS], yh[:, GCOLS : 2 * GCOLS]
            )
            for c in range(1, YC):
                nc.sync.dma_start(yh_sb[:, ts(c, LW)], yh[:, ts(c, LW)])

            # ysq_b[p, j] = ysq16[j], built on the otherwise-idle GpSimd
            # engine in chunks (consumed by the VectorE epilogue paths).
            ysq_b = cpool.tile([P, M], _f16)
            for c in range(YC):
                nc.gpsimd.partition_broadcast(
                    ysq_b[:, ts(c, LW)], ysq_row[0:1, ts(c, LW)]
                )

            def emit_group(mc, gg, ot, h):
                """One [128, GCOLS] PSUM group -> ot[:, h*GCOLS:...]."""
                x_w = xm2_sb[:, ts(mc, P)]
                c0 = gg * GCOLS
                ps = pspool.tile([P, GCOLS], _f32, tag="ps")
                for jj in range(GW):
                    nc.tensor.matmul(
                        ps[:, ts(jj, NT)],
                        x_w,
                        yh_sb[:, c0 + jj * NT : c0 + (jj + 1) * NT],
                        start=True,
                        stop=True,
                    )
                osl = ot[:, h * GCOLS : (h + 1) * GCOLS]
                if (2 * mc + h) % 4 == 0:
                    # Fused VectorE epilogue: out = (psum + xsq) + ysq_b.
                    nc.vector.scalar_tensor_tensor(
                        osl,
                        ps[:],
                        xsq_sb[:, mc : mc + 1],
                        ysq_b[:, c0 : c0 + GCOLS],
                        op0=_ADD,
                        op1=_ADD,
                    )
                else:
                    # ScalarE: a = psum + xsq (frees PSUM), then an all-fp16
                    # 2x-mode VectorE add of the ysq broadcast.
                    a = apool.tile([P, GCOLS], _f16, tag="a")
                    nc.scalar.activation(
                        a[:],
                        ps[:],
                        _IDENT,
                        bias=xsq_sb[:, mc : mc + 1],
                        scale=1.0,
                    )
                    nc.vector.tensor_add(osl, a[:], ysq_b[:, c0 : c0 + GCOLS])

            for g in range(NG):
                for mc in range(MCH):
                    ot = opool.tile([P, SCOLS], _f16, tag="ot")
                    for h in range(2):
                        emit_group(mc, 2 * g + h, ot, h)
                    nc.sync.dma_start(
                        dist16[ts(mc, P), g * SCOLS : (g + 1) * SCOLS], ot[:]
                    )

    nc.compile()
    return nc


def _get_nc():
    global _compiled_nc
    if _compiled_nc is None:
        _compiled_nc = _build()
    return _compiled_nc


def make_in_maps(x: np.ndarray, y: np.ndarray) -> list[dict[str, np.ndarray]]:
    x = np.asarray(x, dtype=np.float32)
    y = np.asarray(y, dtype=np.float32)
    x_sq = np.sum(x * x, axis=1, dtype=np.float32)
    y_sq = np.sum(y * y, axis=1, dtype=np.float32)

    xm2t = (-2.0 * x).T.astype(np.float16)  # [D, N]
    yt16 = np.ascontiguousarray(y.T.astype(np.float16))  # [D, M]
    ysq_in = np.ascontiguousarray(y_sq.astype(np.float16).reshape(1, M))

    in_maps = []
    for c in range(NCORES):
        sl = slice(c * SLAB, (c + 1) * SLAB)
        # [P, MCH]: column mc holds x_sq for rows mc*128..mc*128+127
        xsq_in = np.ascontiguousarray(x_sq[sl].reshape(MCH, P).T)
        in_maps.append(
            {
                "xm2": np.ascontiguousarray(xm2t[:, sl]),
                "yh": yt16,
                "ysq16": ysq_in,
                "xsq": xsq_in,
            }
        )
    return in_maps


def kernel(x: np.ndarray, y: np.ndarray, **run_kwargs) -> np.ndarray:
    nc = _get_nc()
    in_maps = make_in_maps(x, y)
    res = run_bass_kernel_spmd(nc, in_maps, core_ids=list(range(NCORES)), **run_kwargs)
    out = np.concatenate(
        [res.results[c]["dist16"] for c in range(NCORES)], axis=0
    ).astype(np.float32)
    if run_kwargs:
        kernel.last_results = res
    return out
